# revision 12
# baseline (speedup 1.0000x reference)
# Trainium2 Bass kernel for nn_Deep_MT_DGF_GNN (CNN + DGF-GNN + fusion, dueling heads).
# Data-parallel: batch 128 sharded as 16 samples on each of 8 NeuronCores.
# BatchNorm uses exact global batch stats via two small AllReduces.
import sys
sys.path.insert(0, '/opt/trn_rl_repo')
import numpy as np

import concourse.bass as bass
import concourse.bacc as bacc
import concourse.tile as tile
from concourse import mybir
from concourse.bass_utils import run_bass_kernel_spmd

AF = mybir.ActivationFunctionType
ALU = mybir.AluOpType
F32 = mybir.dt.float32
F16 = mybir.dt.float16
AX = mybir.AxisListType

B, N, HD = 128, 256, 64
NC = 8
BS = B // NC          # 16 samples per core
EPS = 1e-5

_STATE = {}


def _dedup_heads(log_sig):
    sig = np.exp(np.asarray(log_sig, np.float64))
    c = -1.0 / (2.0 * sig ** 2 + 1e-6)          # adj_h = exp(c_h * d2)
    H = len(c)
    uniq = {}
    for ch in c:
        uniq[float(ch)] = uniq.get(float(ch), 0) + 1
    # psum holds -d2/2, so adj_h = exp((-2 c_h) * psum); head-mean folds into bias
    return [(-2.0 * ch, float(np.log(cnt / H))) for ch, cnt in uniq.items()]


def _build_program(heads_gnn, heads_fus):
    nc = bacc.Bacc(num_devices=NC)

    def din(name, shape, dt=F32):
        return nc.dram_tensor(name, shape, dt, kind="ExternalInput")

    T = {}
    T["XPAD"] = din("XPAD", [40, 2 * 66 * 66], F16)
    T["STATST"] = din("STATST", [7, BS * N], F32)
    T["PERIT"] = din("PERIT", [55, BS], F32)
    T["W1"] = din("W1", [40, 9 * 128], F16)
    T["W2"] = din("W2", [64, 9 * 128], F16)
    T["SEL16"] = din("SEL16", [128, 16], F32)
    T["SEL32"] = din("SEL32", [128, 32], F32)
    for nm in ["BN1G", "BN1B", "BN2G", "BN2B"]:
        T[nm] = din(nm, [128, 1])
    T["WCP"] = din("WCP", [32, 4 * 64], F16)
    T["CPB"] = din("CPB", [64, 1])
    T["WG"] = din("WG", [7, 64], F32)
    T["GB"] = din("GB", [64, 1])
    for l in range(3):
        T[f"PW{l}"] = din(f"PW{l}", [64, 64], F16)
        T[f"PBM{l}"] = din(f"PBM{l}", [128, 64], F32)
        T[f"LNGM{l}"] = din(f"LNGM{l}", [128, 64], F32)
        T[f"LNBM{l}"] = din(f"LNBM{l}", [128, 64], F32)
    T["WPT"] = din("WPT", [55, 64], F32)
    for nm in ["PBROW", "PLNG", "PLNB", "FPBM", "FLNG", "FLNB"]:
        T[nm] = din(nm, [128, 64], F32)
    T["FW"] = din("FW", [64, 64], F16)
    T["MASK48"] = din("MASK48", [48, 48], F16)
    T["V1T"] = din("V1T", [64, 192], F16); T["V1B"] = din("V1B", [64, 1])
    T["V2T"] = din("V2T", [64, 2], F16);   T["V2B"] = din("V2B", [2, 1])
    T["A1T"] = din("A1T", [64, 192], F16); T["A1B"] = din("A1B", [64, 1])
    T["A2T"] = din("A2T", [64, 2], F16);   T["A2B"] = din("A2B", [2, 1])
    T["ID32"] = din("ID32", [128, 128], F32)
    T["ID16"] = din("ID16", [128, 128], F16)
    T["OV"] = nc.dram_tensor("OV", [BS, 2], F32, kind="ExternalOutput")
    T["OA"] = nc.dram_tensor("OA", [BS, 2], F32, kind="ExternalOutput")
    T["cc1_in"] = nc.dram_tensor("cc1_in", [16, 2], F32)
    T["cc1_out"] = nc.dram_tensor("cc1_out", [16, 2], F32, addr_space="Shared")
    T["cc2_in"] = nc.dram_tensor("cc2_in", [32, 2], F32)
    T["cc2_out"] = nc.dram_tensor("cc2_out", [32, 2], F32, addr_space="Shared")
    T["RG"] = [list(range(NC))]

    with tile.TileContext(nc) as tc:
        _emit(nc, tc, T, heads_gnn, heads_fus)
    nc.compile()
    return nc


def _emit(nc, tc, T, heads_gnn, heads_fus):
    from contextlib import ExitStack
    ctx = ExitStack()
    with ctx:
        const = ctx.enter_context(tc.tile_pool(name="const", bufs=1))
        gnnp = ctx.enter_context(tc.tile_pool(name="gnnp", bufs=1))

        def load(dram):
            t = const.tile(list(dram.shape), dram.dtype, tag=dram.name, name=dram.name)
            nc.sync.dma_start(out=t, in_=dram[tuple(slice(None) for _ in dram.shape)])
            return t

        L = {k: load(T[k]) for k in T
             if k not in ("OV", "OA", "cc1_in", "cc1_out", "cc2_in", "cc2_out", "RG",
                          "XPAD", "STATST")}
        id32, id16 = L["ID32"], L["ID16"]

        epsT = const.tile([128, 1], F32, name="epsT")
        nc.vector.memset(epsT, EPS)

        def ptr(out, in_):
            idt = id32 if in_.dtype == F32 else id16
            nc.tensor.transpose(out, in_, idt[: in_.shape[0], : in_.shape[0]])

        # transpose warmups (absorb identity deps on PE early)
        with tc.tile_pool(name="wps", bufs=1, space="PSUM") as wpool:
            wm32 = wpool.tile([1, 128], F32)
            nc.tensor.transpose(wm32, id32[:, 0:1], id32)
            wm16 = wpool.tile([1, 128], F16)
            nc.tensor.transpose(wm16, id16[:, 0:1], id16)

        def elu_inline(dst, src, tmp_pool):
            emin = tmp_pool.tile(list(src.shape), F32, tag="elu_min")
            nc.gpsimd.tensor_scalar(out=emin, in0=src, scalar1=0.0, scalar2=None,
                                    op0=ALU.min)
            eexp = tmp_pool.tile(list(src.shape), F32, tag="elu_exp")
            nc.scalar.activation(out=eexp, in_=emin, func=AF.Exp)
            emax = tmp_pool.tile(list(src.shape), F32, tag="elu_min", name="emax")
            nc.gpsimd.tensor_scalar(out=emax, in0=src, scalar1=0.0, scalar2=None,
                                    op0=ALU.max)
            nc.vector.scalar_tensor_tensor(out=dst, in0=emax, scalar=-1.0,
                                           in1=eexp, op0=ALU.add, op1=ALU.add)

        def layernorm64(lp, work, res, lng, lnb, n_groups, want_T,
                        g_out=None, gT_out=None):
            """res [P, n_groups*64] -> (g_new, gT_new or None); P=128 assumed for T."""
            P = res.shape[0]
            cols = n_groups * 64
            rv = res.rearrange("p (t d) -> p t d", d=64)
            mean = lp.tile([P, n_groups], F32, tag="ln_mean")
            nc.vector.tensor_reduce(out=mean, in_=rv, axis=AX.X, op=ALU.add)
            nc.vector.tensor_scalar_mul(mean, mean, 1.0 / 64.0)
            sscr = lp.tile([P, cols], F32, tag="scrA")
            nc.gpsimd.tensor_tensor(out=sscr, in0=res, in1=res, op=ALU.mult)
            msq = lp.tile([P, n_groups], F32, tag="ln_msq")
            nc.vector.tensor_reduce(out=msq, in_=sscr.rearrange("p (t d) -> p t d", d=64),
                                    axis=AX.X, op=ALU.add)
            nc.vector.tensor_scalar_mul(msq, msq, 1.0 / 64.0)
            m2 = lp.tile([P, n_groups], F32, tag="ln_m2")
            nc.vector.tensor_mul(m2, mean, mean)
            var = lp.tile([P, n_groups], F32, tag="ln_var")
            nc.vector.tensor_sub(var, msq, m2)
            rstd = lp.tile([P, n_groups], F32, tag="ln_rstd")
            nc.scalar.activation(out=rstd, in_=var, func=AF.Sqrt, bias=epsT[:P, :])
            nc.vector.reciprocal(rstd, rstd)
            xhat = lp.tile([P, cols], F32, tag="scrB")
            for t in range(n_groups):
                nc.vector.tensor_scalar(out=xhat[:, 64 * t:64 * (t + 1)],
                                        in0=res[:, 64 * t:64 * (t + 1)],
                                        scalar1=mean[:, t:t + 1], scalar2=rstd[:, t:t + 1],
                                        op0=ALU.subtract, op1=ALU.mult)
            scaled = lp.tile([P, cols], F32, tag="ghi", name="scaled")
            lngr = bass.AP(tensor=lng.tensor, offset=lng.offset,
                           ap=[[lng.ap[0][0], P], [0, n_groups], lng.ap[1]])
            nc.gpsimd.tensor_tensor(out=scaled.rearrange("p (t d) -> p t d", d=64),
                                    in0=xhat.rearrange("p (t d) -> p t d", d=64),
                                    in1=lngr, op=ALU.mult)
            g_new = g_out if g_out is not None else lp.tile([P, cols], F32, tag="ln_out", name="ln_out")
            lnbr = bass.AP(tensor=lnb.tensor, offset=lnb.offset,
                           ap=[[lnb.ap[0][0], P], [0, n_groups], lnb.ap[1]])
            nc.gpsimd.tensor_tensor(out=g_new.rearrange("p (t d) -> p t d", d=64),
                                    in0=scaled.rearrange("p (t d) -> p t d", d=64),
                                    in1=lnbr, op=ALU.add)
            gT_new = None
            if want_T:
                gT_new = gT_out if gT_out is not None else lp.tile(
                    [64, n_groups * 128], F32, tag="ln_gT", name="ln_gT")
                with tc.tile_pool(name="lnTps", bufs=3, space="PSUM") as lnps:
                    for t in range(n_groups):
                        pt = lnps.tile([64, 128], F32, tag="lnT")
                        ptr(pt, g_new[:, 64 * t:64 * (t + 1)])
                        nc.vector.tensor_copy(gT_new[:, 128 * t:128 * (t + 1)], pt)
            return g_new, gT_new

        # ============ conv1: y1 [128, 8192], p = 16*blk + co ============
        y1pool = ctx.enter_context(tc.tile_pool(name="y1p", bufs=1))
        y1 = y1pool.tile([128, 8192], F32)
        c1parts = y1pool.tile([128, 16, 2], F32)
        w1 = L["W1"]
        with tc.tile_pool(name="xpadp", bufs=1) as xpadp, \
             tc.tile_pool(name="c1ps", bufs=2, space="PSUM") as c1ps, \
             tc.tile_pool(name="c1scr", bufs=2) as c1scr:
            xpad = xpadp.tile(list(T["XPAD"].shape), F16)
            nc.sync.dma_start(out=xpad, in_=T["XPAD"][:, :])
            xpv = xpad.rearrange("p (s y x) -> p s y x", s=2, y=66)
            for q in range(16):
                st, r = q // 8, q % 8
                pg = c1ps.tile([128, 512], F32, tag="pg")
                for t in range(9):
                    dy, dx = t // 3, t % 3
                    rhs = xpv[:, st, dy + 8 * r: dy + 8 * r + 8, dx: dx + 64]
                    nc.tensor.matmul(pg, w1[:, 128 * t:128 * (t + 1)], rhs,
                                     start=(t == 0), stop=(t == 8))
                nc.scalar.activation(out=y1[:, 512 * q:512 * (q + 1)], in_=pg,
                                     func=AF.Copy, accum_out=c1parts[:, q, 0:1])
                scr = c1scr.tile([128, 512], F32, tag="scr")
                nc.scalar.activation(out=scr, in_=pg, func=AF.Square,
                                     accum_out=c1parts[:, q, 1:2])

        with tc.tile_pool(name="bnfold1", bufs=1) as bnf, \
             tc.tile_pool(name="bnps1", bufs=1, space="PSUM") as bnps:
            part1 = bnf.tile([128, 2], F32)
            nc.vector.tensor_reduce(out=part1, in_=c1parts.rearrange('p q c -> p c q'), axis=AX.X, op=ALU.add)
            ps1 = bnps.tile([16, 2], F32)
            nc.tensor.matmul(ps1, L["SEL16"], part1, start=True, stop=True)
            sb1 = bnf.tile([16, 2], F32)
            nc.vector.tensor_copy(sb1, ps1)
            nc.sync.dma_start(out=T["cc1_in"][:, :], in_=sb1)
        nc.gpsimd.collective_compute("AllReduce", ALU.add, replica_groups=T["RG"],
                                     ins=[T["cc1_in"][:, :]], outs=[T["cc1_out"][:, :]])

        # ============ GNN init ============
        gT0 = gnnp.tile([64, 4096], F32, tag="gT_pp0")
        g0 = gnnp.tile([128, 2048], F32, tag="g_pp0")
        with tc.tile_pool(name="g0ps", bufs=2, space="PSUM") as g0ps, \
             tc.tile_pool(name="stp", bufs=1) as stp:
            statsT = stp.tile(list(T["STATST"].shape), F32)
            nc.sync.dma_start(out=statsT, in_=T["STATST"][:, :])
            for q in range(8):
                pg = g0ps.tile([64, 512], F32, tag="g0")
                nc.tensor.matmul(pg, L["WG"], statsT[:, 512 * q:512 * (q + 1)],
                                 start=True, stop=True)
                nc.scalar.activation(out=gT0[:, 512 * q:512 * (q + 1)], in_=pg,
                                     func=AF.Identity, bias=L["GB"], scale=1.0)
        with tc.tile_pool(name="t0ps", bufs=3, space="PSUM") as t0ps:
            for t in range(32):
                pt = t0ps.tile([128, 64], F32, tag="t0")
                ptr(pt, gT0[:, 128 * t:128 * (t + 1)])
                nc.vector.tensor_copy(g0[:, 64 * t:64 * (t + 1)], pt)

        def gnn_layer(lidx, gT, g_node, heads, want_T):
            with tc.tile_pool(name=f"l{lidx}", bufs=1) as lp, \
                 tc.tile_pool(name=f"l{lidx}adj", bufs=4) as adjp, \
                 tc.tile_pool(name=f"l{lidx}w", bufs=1) as wk:
                ghi = lp.tile([64, 4096], F16, tag="ghi")
                nc.vector.tensor_copy(ghi, gT)
                glo = lp.tile([64, 4096], F16, tag="glo")
                nc.vector.tensor_sub(glo, gT, ghi)
                gpT16 = lp.tile([64, 4096], F16, tag="gpT16")
                gp_node = lp.tile([128, 2048], F16, tag="gp_node")
                with tc.tile_pool(name=f"l{lidx}tp", bufs=2, space="PSUM") as tps:
                    for q in range(8):
                        pg = tps.tile([64, 512], F32, tag="gp")
                        nc.tensor.matmul(pg, L[f"PW{lidx}"], ghi[:, 512 * q:512 * (q + 1)],
                                         start=True, stop=True)
                        nc.scalar.activation(out=gpT16[:, 512 * q:512 * (q + 1)], in_=pg,
                                             func=AF.Copy)
                    for t in range(32):
                        pt = tps.tile([128, 64], F16, tag="tp16")
                        ptr(pt, gpT16[:, 128 * t:128 * (t + 1)])
                        nc.scalar.activation(out=gp_node[:, 64 * t:64 * (t + 1)],
                                             in_=pt, func=AF.Copy)
                # sq (partition-major); l2big row0 = -sq/2 free-major; o2big row1 same
                sqscr = lp.tile([128, 2048], F32, tag="scrA", name="sqscr")
                nc.gpsimd.tensor_tensor(out=sqscr, in0=g_node, in1=g_node, op=ALU.mult)
                sqp = lp.tile([128, 32], F32, tag="sqp")
                nc.vector.tensor_reduce(out=sqp,
                                        in_=sqscr.rearrange("p (t d) -> p t d", d=64),
                                        axis=AX.X, op=ALU.add)
                l2big = lp.tile([2, 4096], F32, tag="scrA", name="l2big")
                nc.vector.memset(l2big, 1.0)
                with tc.tile_pool(name=f"l{lidx}sr", bufs=3, space="PSUM") as srp:
                    for t in range(32):
                        pt = srp.tile([1, 128], F32, tag="sr")
                        ptr(pt, sqp[:, t:t + 1])
                        nc.scalar.mul(l2big[0:1, 128 * t:128 * (t + 1)], pt, -0.5)
                o2big = lp.tile([2, 4096], F32, tag="scrB", name="o2big")
                nc.vector.memset(o2big, 1.0)
                nc.sync.dma_start(out=o2big[1:2, :], in_=l2big[0:1, :])

                msgS = lp.tile([128, 2048], F32, tag="msgS")
                lps_cm = tc.tile_pool(name=f"l{lidx}ps", bufs=2, space="PSUM")
                lps = lps_cm.__enter__()
                for s in range(16):
                    adj = [None, None]
                    for kn in range(2):
                        pgr = lps.tile([128, 256], F32, tag="gram")
                        lo_ = 256 * s + 128 * kn
                        nc.tensor.matmul(pgr, ghi[:, lo_:lo_ + 128],
                                         ghi[:, 256 * s:256 * s + 256],
                                         start=True, stop=False)
                        nc.tensor.matmul(pgr, ghi[:, lo_:lo_ + 128],
                                         glo[:, 256 * s:256 * s + 256],
                                         start=False, stop=False)
                        nc.tensor.matmul(pgr, glo[:, lo_:lo_ + 128],
                                         ghi[:, 256 * s:256 * s + 256],
                                         start=False, stop=False)
                        nc.tensor.matmul(pgr, l2big[:, lo_:lo_ + 128],
                                         o2big[:, 256 * s:256 * s + 256],
                                         start=False, stop=True, skip_group_check=True)
                        sc0, b0 = heads[0]
                        at = adjp.tile([128, 256], F16, tag="adj")
                        nc.scalar.activation(out=at, in_=pgr, func=AF.Exp,
                                             scale=float(sc0), bias=float(b0))
                        for (sch, bh) in heads[1:]:
                            ath = adjp.tile([128, 256], F16, tag="adjh")
                            nc.scalar.activation(out=ath, in_=pgr, func=AF.Exp,
                                                 scale=float(sch), bias=float(bh))
                            nc.vector.tensor_add(at, at, ath)
                        adj[kn] = at
                    for kn in range(2):
                        pm = lps.tile([128, 64], F32, tag="msg")
                        for km in range(2):
                            nc.tensor.matmul(pm, adj[km][:, 128 * kn:128 * (kn + 1)],
                                             gp_node[:, 64 * (2 * s + km):
                                                     64 * (2 * s + km) + 64],
                                             start=(km == 0), stop=(km == 1))
                        cc = 64 * (2 * s + kn)
                        nc.vector.tensor_add(msgS[:, cc:cc + 64], pm, L[f"PBM{lidx}"])
                lps_cm.__exit__(None, None, None)
                eluS = lp.tile([128, 2048], F32, tag="gpT16", name="eluS")
                elu_inline(eluS, msgS, wk)
                res = lp.tile([128, 2048], F32, tag="glo", name="res")
                nc.gpsimd.tensor_tensor(out=res, in0=eluS, in1=g_node, op=ALU.add)
                pp = (lidx + 1) % 2
                g_new = gnnp.tile([128, 2048], F32, tag=f"g_pp{pp}", name=f"gn{lidx}")
                if want_T:
                    gT_new = gnnp.tile([64, 4096], F32, tag=f"gT_pp{pp}", name=f"gTn{lidx}")
                else:
                    gT_new = None
                layernorm64(lp, wk, res, L[f"LNGM{lidx}"], L[f"LNBM{lidx}"],
                            32, want_T, g_new, gT_new)
                return gT_new, g_new

        gT1, g1 = gnn_layer(0, gT0, g0, heads_gnn[0], True)

        # ============ BN1 apply + maxpool + elu ============
        u1pool = ctx.enter_context(tc.tile_pool(name="u1p", bufs=1))
        u1 = u1pool.tile([128, 2048], F16)
        with tc.tile_pool(name="bn1", bufs=1) as bp:
            rep1 = bp.tile([128, 2], F32)
            src = bass.AP(tensor=T["cc1_out"][:, :].tensor, offset=0,
                          ap=[[0, 8], [2, 16], [1, 2]])
            nc.sync.dma_start(out=rep1, in_=src)
            CNT1 = float(B) * 64 * 64
            mu = bp.tile([128, 1], F32)
            nc.vector.tensor_scalar_mul(mu, rep1[:, 0:1], 1.0 / CNT1)
            va = bp.tile([128, 1], F32)
            nc.vector.tensor_scalar_mul(va, rep1[:, 1:2], 1.0 / CNT1)
            m2b = bp.tile([128, 1], F32)
            nc.vector.tensor_mul(m2b, mu, mu)
            nc.vector.tensor_sub(va, va, m2b)
            nc.scalar.activation(out=va, in_=va, func=AF.Sqrt, bias=epsT)
            nc.vector.reciprocal(va, va)
            gam = bp.tile([128, 1], F32)
            nc.vector.tensor_mul(gam, L["BN1G"], va)
            bet = bp.tile([128, 1], F32)
            nc.vector.tensor_mul(bet, gam, mu)
            nc.vector.tensor_sub(bet, L["BN1B"], bet)
            z1 = bp.tile([128, 8192], F32)
            nc.vector.tensor_scalar(out=z1, in0=y1, scalar1=gam, scalar2=bet,
                                    op0=ALU.mult, op1=ALU.add)
            zv = z1.rearrange("p (s y x) -> p s y x", s=2, y=64)
            m1 = bp.tile([128, 2, 64, 32], F32)
            nc.vector.tensor_tensor(out=m1, in0=zv[:, :, :, 0::2],
                                    in1=zv[:, :, :, 1::2], op=ALU.max)
            m2p = bp.tile([128, 2, 32, 32], F32)
            nc.vector.tensor_tensor(out=m2p, in0=m1[:, :, 0::2, :],
                                    in1=m1[:, :, 1::2, :], op=ALU.max)
            m2f = m2p.rearrange("p s y x -> p (s y x)")
            with tc.tile_pool(name="elu1", bufs=1) as ep:
                elu_inline(u1, m2f, ep)

        # ============ conv2: y2 [128, 4096], p = 32*b4 + co ============
        y2pool = ctx.enter_context(tc.tile_pool(name="y2p", bufs=1))
        y2 = y2pool.tile([128, 4096], F32)
        c2parts = y2pool.tile([128, 8, 2], F32)
        w2 = L["W2"]
        with tc.tile_pool(name="xp2", bufs=1) as xp2p:
            xpad2 = xp2p.tile([64, 4, 34, 34], F16)
            nc.gpsimd.memset(xpad2, 0.0)
            u1v = u1.rearrange("p (s y x) -> p s y x", s=2, y=32)
            for s4 in range(4):
                for b4 in range(4):
                    nc.sync.dma_start(
                        out=xpad2[16 * b4:16 * b4 + 16, s4, 1:33, 1:33],
                        in_=u1v[32 * b4 + 16 * (s4 // 2):32 * b4 + 16 * (s4 // 2) + 16,
                                s4 % 2])
            with tc.tile_pool(name="c2ps", bufs=2, space="PSUM") as c2ps, \
                 tc.tile_pool(name="c2scr", bufs=2) as c2scr:
                for q in range(8):
                    s4, r2 = q // 2, q % 2
                    pg = c2ps.tile([128, 512], F32, tag="pg2")
                    for t in range(9):
                        dy, dx = t // 3, t % 3
                        rhs = xpad2[:, s4, dy + 16 * r2: dy + 16 * r2 + 16, dx: dx + 32]
                        nc.tensor.matmul(pg, w2[:, 128 * t:128 * (t + 1)], rhs,
                                         start=(t == 0), stop=(t == 8))
                    nc.scalar.activation(out=y2[:, 512 * q:512 * (q + 1)], in_=pg,
                                         func=AF.Copy, accum_out=c2parts[:, q, 0:1])
                    scr = c2scr.tile([128, 512], F32, tag="scr2")
                    nc.scalar.activation(out=scr, in_=pg, func=AF.Square,
                                         accum_out=c2parts[:, q, 1:2])
        with tc.tile_pool(name="bnfold2", bufs=1) as bnf, \
             tc.tile_pool(name="bnps2", bufs=1, space="PSUM") as bnps:
            part2 = bnf.tile([128, 2], F32)
            nc.vector.tensor_reduce(out=part2, in_=c2parts.rearrange('p q c -> p c q'), axis=AX.X, op=ALU.add)
            ps2 = bnps.tile([32, 2], F32)
            nc.tensor.matmul(ps2, L["SEL32"], part2, start=True, stop=True)
            sb2 = bnf.tile([32, 2], F32)
            nc.vector.tensor_copy(sb2, ps2)
            nc.sync.dma_start(out=T["cc2_in"][:, :], in_=sb2)
        nc.gpsimd.collective_compute("AllReduce", ALU.add, replica_groups=T["RG"],
                                     ins=[T["cc2_in"][:, :]], outs=[T["cc2_out"][:, :]])

        gT2, g2 = gnn_layer(1, gT1, g1, heads_gnn[1], True)
        gT3, g3 = gnn_layer(2, gT2, g2, heads_gnn[2], True)

        hgnnT = gnnp.tile([64, 16], F32, tag="hgnnT")
        nc.vector.tensor_reduce(out=hgnnT, in_=gT3.rearrange("p (s n) -> p s n", n=256),
                                axis=AX.X, op=ALU.add)
        nc.vector.tensor_scalar_mul(hgnnT, hgnnT, 1.0 / 256.0)

        # ============ BN2 apply + elu + avgpool + cnn proj ============
        hcnnT = gnnp.tile([64, 16], F32, tag="hcnnT")
        with tc.tile_pool(name="bn2", bufs=1) as bp:
            rep2 = bp.tile([128, 2], F32)
            src = bass.AP(tensor=T["cc2_out"][:, :].tensor, offset=0,
                          ap=[[0, 4], [2, 32], [1, 2]])
            nc.sync.dma_start(out=rep2, in_=src)
            CNT2 = float(B) * 32 * 32
            mu = bp.tile([128, 1], F32)
            nc.vector.tensor_scalar_mul(mu, rep2[:, 0:1], 1.0 / CNT2)
            va = bp.tile([128, 1], F32)
            nc.vector.tensor_scalar_mul(va, rep2[:, 1:2], 1.0 / CNT2)
            m2b = bp.tile([128, 1], F32)
            nc.vector.tensor_mul(m2b, mu, mu)
            nc.vector.tensor_sub(va, va, m2b)
            nc.scalar.activation(out=va, in_=va, func=AF.Sqrt, bias=epsT)
            nc.vector.reciprocal(va, va)
            gam = bp.tile([128, 1], F32)
            nc.vector.tensor_mul(gam, L["BN2G"], va)
            bet = bp.tile([128, 1], F32)
            nc.vector.tensor_mul(bet, gam, mu)
            nc.vector.tensor_sub(bet, L["BN2B"], bet)
            z2 = bp.tile([128, 4096], F32)
            nc.vector.tensor_scalar(out=z2, in0=y2, scalar1=gam, scalar2=bet,
                                    op0=ALU.mult, op1=ALU.add)
            e2 = bp.tile([128, 4096], F32)
            with tc.tile_pool(name="elu2", bufs=1) as ep:
                elu_inline(e2, z2, ep)
            r1 = bp.tile([128, 8, 16, 2], F32)
            nc.vector.tensor_reduce(out=r1,
                                    in_=e2.rearrange("p (c y q x) -> p c y q x",
                                                     c=8, y=16, q=2),
                                    axis=AX.X, op=ALU.add)
            flat16 = bp.tile([128, 16], F16)
            r1v = bass.AP(tensor=r1.tensor, offset=r1.offset,
                          ap=[r1.ap[0], [32, 8], [1, 2], [2, 16]])
            with nc.allow_low_precision(reason="pooled sums fit f16"):
                nc.vector.tensor_reduce(out=flat16.rearrange("p (c q) -> p c q", c=8),
                                        in_=r1v, axis=AX.X, op=ALU.add)
            flat4 = bp.tile([32, 64], F16)
            for b4 in range(4):
                nc.sync.dma_start(out=flat4[:, 16 * b4:16 * b4 + 16],
                                  in_=flat16[32 * b4:32 * b4 + 32, :])
            f4v = flat4.rearrange("p (a c) -> p a c", a=4)
            with tc.tile_pool(name="hcps", bufs=1, space="PSUM") as hps:
                phc = hps.tile([64, 16], F32)
                for b4 in range(4):
                    for i in range(4):
                        rhs = f4v[:, b4, i::4]
                        nc.tensor.matmul(phc[:, 4 * b4:4 * (b4 + 1)],
                                         L["WCP"][:, 64 * i:64 * (i + 1)], rhs,
                                         start=(i == 0), stop=(i == 3))
                nc.vector.tensor_scalar(out=hcnnT, in0=phc, scalar1=L["CPB"],
                                        scalar2=None, op0=ALU.add)

        # ============ peri ============
        hperiT = gnnp.tile([64, 16], F32, tag="hperiT")
        with tc.tile_pool(name="peri", bufs=1) as pp, \
             tc.tile_pool(name="pps", bufs=2, space="PSUM") as pps:
            php = pps.tile([16, 64], F32, tag="p1")
            nc.tensor.matmul(php, L["PERIT"], L["WPT"], start=True, stop=True)
            hp = pp.tile([16, 64], F32)
            nc.vector.tensor_add(hp, php, L["PBROW"][:16, :])
            hpn, _ = layernorm64(pp, pp, hp, L["PLNG"], L["PLNB"], 1, False)
            hpe = pp.tile([16, 64], F32)
            with tc.tile_pool(name="elu3", bufs=1) as ep:
                elu_inline(hpe, hpn, ep)
            pt = pps.tile([64, 16], F32, tag="pT")
            ptr(pt, hpe)
            nc.vector.tensor_copy(hperiT, pt)

        # ============ fusion + heads ============
        with tc.tile_pool(name="fus", bufs=1) as fp:
            fps_cm = tc.tile_pool(name="fpsA", bufs=1, space="PSUM")
            fps = fps_cm.__enter__()
            combT = fp.tile([64, 48], F32)
            cv = combT.rearrange("p (s t) -> p s t", t=3)
            nc.vector.tensor_copy(cv[:, :, 0], hcnnT)
            nc.vector.tensor_copy(cv[:, :, 1], hgnnT)
            nc.vector.tensor_copy(cv[:, :, 2], hperiT)
            pcn = fps.tile([48, 64], F32, tag="cn")
            ptr(pcn, combT)
            comb = fp.tile([48, 64], F32)
            nc.vector.tensor_copy(comb, pcn)
            scr = fp.tile([48, 64], F32)
            nc.vector.tensor_mul(scr, comb, comb)
            sq48 = fp.tile([48, 1], F32)
            nc.vector.tensor_reduce(out=sq48, in_=scr, axis=AX.X, op=ALU.add)
            psr = fps.tile([1, 48], F32, tag="sr48")
            ptr(psr, sq48)
            srow = fp.tile([1, 48], F32)
            nc.scalar.mul(srow, psr, -0.5)
            o2 = fp.tile([2, 48], F32)
            nc.vector.memset(o2, 1.0)
            nc.sync.dma_start(out=o2[1:2, :], in_=srow)
            l2 = fp.tile([2, 48], F32)
            nc.vector.memset(l2, 1.0)
            nc.sync.dma_start(out=l2[0:1, :], in_=srow)
            chi = fp.tile([64, 48], F16)
            nc.vector.tensor_copy(chi, combT)
            clo = fp.tile([64, 48], F16)
            nc.vector.tensor_sub(clo, combT, chi)
            pgr = fps.tile([48, 48], F32, tag="gr48")
            nc.tensor.matmul(pgr, chi, chi, start=True, stop=False)
            nc.tensor.matmul(pgr, chi, clo, start=False, stop=False)
            nc.tensor.matmul(pgr, clo, chi, start=False, stop=False)
            nc.tensor.matmul(pgr, l2, o2, start=False, stop=True, skip_group_check=True)
            sc0, b0 = heads_fus[0]
            adjf = fp.tile([48, 48], F32)
            nc.scalar.activation(out=adjf, in_=pgr, func=AF.Exp,
                                 scale=float(sc0), bias=float(b0))
            for (sch, bh) in heads_fus[1:]:
                ah = fp.tile([48, 48], F32, tag="ah")
                nc.scalar.activation(out=ah, in_=pgr, func=AF.Exp,
                                     scale=float(sch), bias=float(bh))
                nc.vector.tensor_add(adjf, adjf, ah)
            adjm = fp.tile([48, 48], F16)
            nc.vector.tensor_mul(adjm, adjf, L["MASK48"])
            pgp = fps.tile([64, 48], F32, tag="gp48")
            nc.tensor.matmul(pgp, L["FW"], chi, start=True, stop=True)
            gpT = fp.tile([64, 48], F16)
            nc.vector.tensor_copy(gpT, pgp)
            pgn = fps.tile([48, 64], F16, tag="gpn48")
            ptr(pgn, gpT)
            gpn = fp.tile([48, 64], F16)
            nc.vector.tensor_copy(gpn, pgn)
            fps_cm.__exit__(None, None, None)
            fps_cm2 = tc.tile_pool(name="fpsB", bufs=1, space="PSUM")
            fps = fps_cm2.__enter__()
            pmsg = fps.tile([48, 64], F32, tag="msg48")
            nc.tensor.matmul(pmsg, adjm, gpn, start=True, stop=True)
            msgf = fp.tile([48, 64], F32)
            nc.vector.tensor_add(msgf, pmsg, L["FPBM"][:48, :])
            eluf = fp.tile([48, 64], F32)
            with tc.tile_pool(name="eluf", bufs=1) as ep:
                elu_inline(eluf, msgf, ep)
            resf = fp.tile([48, 64], F32)
            nc.vector.tensor_add(resf, eluf, comb)
            fused, _ = layernorm64(fp, fp, resf, L["FLNG"], L["FLNB"], 1, False)
            pft = fps.tile([64, 48], F32, tag="ft")
            ptr(pft, fused)
            fusedT = fp.tile([64, 48], F16)
            nc.vector.tensor_copy(fusedT, pft)
            ftv = fusedT.rearrange("p (s t) -> p s t", t=3)
            for (w1t, b1t, w2t, b2t, OD) in [
                    (L["V1T"], L["V1B"], L["V2T"], L["V2B"], T["OV"]),
                    (L["A1T"], L["A1B"], L["A2T"], L["A2B"], T["OA"])]:
                ph = fps.tile([64, 16], F32, tag="h1")
                for t in range(3):
                    nc.tensor.matmul(ph, w1t[:, 64 * t:64 * (t + 1)], ftv[:, :, t],
                                     start=(t == 0), stop=(t == 2))
                hb = fp.tile([64, 16], F32, tag="hb")
                nc.vector.tensor_scalar(out=hb, in0=ph, scalar1=b1t, scalar2=None,
                                        op0=ALU.add)
                he = fp.tile([64, 16], F16, tag="he")
                with tc.tile_pool(name="eluh" + OD.name, bufs=1) as ep:
                    elu_inline(he, hb, ep)
                p2 = fps.tile([2, 16], F32, tag="h2")
                nc.tensor.matmul(p2, w2t, he, start=True, stop=True)
                ob = fp.tile([2, 16], F32, tag="ob")
                nc.vector.tensor_scalar(out=ob, in0=p2, scalar1=b2t, scalar2=None,
                                        op0=ALU.add)
                nc.sync.dma_start(out=OD[:, :].rearrange("s k -> k s"), in_=ob)
            fps_cm2.__exit__(None, None, None)


def _prep_weights(params):
    p = params
    c1 = np.asarray(p['conv1_w'], np.float32)
    w1 = np.zeros((40, 9 * 128), np.float16)
    for t in range(9):
        dy, dx = t // 3, t % 3
        for blk in range(8):
            w1[5 * blk:5 * blk + 5, 128 * t + 16 * blk:128 * t + 16 * blk + 16] = \
                c1[:, :, dy, dx].T
    c2 = np.asarray(p['conv2_w'], np.float32)
    w2 = np.zeros((64, 9 * 128), np.float16)
    for t in range(9):
        dy, dx = t // 3, t % 3
        for blk in range(4):
            w2[16 * blk:16 * blk + 16, 128 * t + 32 * blk:128 * t + 32 * blk + 32] = \
                c2[:, :, dy, dx].T
    sel16 = np.zeros((128, 16), np.float32)
    sel16[np.arange(128), np.arange(128) % 16] = 1
    sel32 = np.zeros((128, 32), np.float32)
    sel32[np.arange(128), np.arange(128) % 32] = 1
    rep = lambda v, m: np.ascontiguousarray(
        np.asarray(v, np.float32)[np.arange(128) % m][:, None])
    rowmat = lambda v: np.ascontiguousarray(
        np.tile(np.asarray(v, np.float32)[None, :], (128, 1)))
    col = lambda v: np.ascontiguousarray(np.asarray(v, np.float32)[:, None])
    cw, cb = p['cnn_proj']
    cw = np.asarray(cw, np.float32)          # [64, 128] cols = (co, qy, qx)
    wcp = np.zeros((32, 4 * 64), np.float16)
    for i in range(4):
        qy, qx = i // 2, i % 2
        for co in range(32):
            wcp[co, 64 * i:64 * (i + 1)] = cw[:, co * 4 + qy * 2 + qx] / 256.0
    gw, gb = p['gnn_map']
    pv, pb_ = p['peri']
    v1w, v1bv = p['v1']; v2w, v2bv = p['v2']
    a1w, a1bv = p['a1']; a2w, a2bv = p['a2']
    v1t = np.ascontiguousarray(np.asarray(v1w, np.float32).T).astype(np.float16)
    a1t = np.ascontiguousarray(np.asarray(a1w, np.float32).T).astype(np.float16)
    # cols of v1t: [64t+j] grouped per token already (d-major rows) -> reorder to [d, 64t+j]
    v1t2 = np.zeros((64, 192), np.float16)
    a1t2 = np.zeros((64, 192), np.float16)
    v1wf = np.asarray(v1w, np.float32)
    a1wf = np.asarray(a1w, np.float32)
    for t in range(3):
        v1t2[:, 64 * t:64 * (t + 1)] = v1wf[:, 64 * t:64 * (t + 1)].T
        a1t2[:, 64 * t:64 * (t + 1)] = a1wf[:, 64 * t:64 * (t + 1)].T
    mask48 = np.zeros((48, 48), np.float16)
    for s in range(16):
        mask48[3 * s:3 * s + 3, 3 * s:3 * s + 3] = 1
    fw_, fb_ = p['fus_proj']
    W = {
        'W1': w1, 'W2': w2, 'SEL16': sel16, 'SEL32': sel32,
        'BN1G': rep(p['bn1_g'], 16), 'BN1B': rep(p['bn1_b'], 16),
        'BN2G': rep(p['bn2_g'], 32), 'BN2B': rep(p['bn2_b'], 32),
        'WCP': wcp, 'CPB': col(cb),
        'WG': np.ascontiguousarray(np.asarray(gw, np.float32).T), 'GB': col(gb),
        'WPT': np.ascontiguousarray(np.asarray(pv, np.float32).T),
        'PBROW': rowmat(pb_), 'PLNG': rowmat(p['peri_ln_g']),
        'PLNB': rowmat(p['peri_ln_b']),
        'FW': np.ascontiguousarray(np.asarray(fw_, np.float32).T).astype(np.float16),
        'FPBM': rowmat(fb_), 'FLNG': rowmat(p['fus_ln_g']), 'FLNB': rowmat(p['fus_ln_b']),
        'MASK48': mask48,
        'V1T': v1t2, 'V1B': col(v1bv),
        'V2T': np.ascontiguousarray(np.asarray(v2w, np.float32).T).astype(np.float16),
        'V2B': col(v2bv),
        'A1T': a1t2, 'A1B': col(a1bv),
        'A2T': np.ascontiguousarray(np.asarray(a2w, np.float32).T).astype(np.float16),
        'A2B': col(a2bv),
        'ID32': np.eye(128, dtype=np.float32), 'ID16': np.eye(128, dtype=np.float16),
    }
    for l in range(3):
        d = p['dgf'][l]
        prw, prb = d['proj']
        W[f'PW{l}'] = np.ascontiguousarray(np.asarray(prw, np.float32).T).astype(np.float16)
        W[f'PBM{l}'] = rowmat(prb)
        W[f'LNGM{l}'] = rowmat(d['ln_g'])
        W[f'LNBM{l}'] = rowmat(d['ln_b'])
    return W


def kernel(maps, stats, peri, params):
    maps = np.asarray(maps, np.float32)
    stats = np.asarray(stats, np.float32)
    peri = np.asarray(peri, np.float32)
    heads_gnn = [_dedup_heads(params['dgf'][l]['log_sig']) for l in range(3)]
    heads_fus = _dedup_heads(params['fus_log_sig'])
    key = (tuple(tuple(h) for h in (tuple(x) for x in map(tuple, heads_gnn))),
           tuple(heads_fus))
    if _STATE.get('key') != key:
        _STATE['nc'] = _build_program(heads_gnn, heads_fus)
        _STATE['key'] = key
    nc = _STATE['nc']
    W = _prep_weights(params)
    in_maps = []
    for c in range(NC):
        s0 = c * BS
        mp = maps[s0:s0 + BS]
        xp = np.zeros((8, 5, 2, 66, 66), np.float16)
        for blk in range(8):
            for j in range(2):
                xp[blk, :, j, 1:65, 1:65] = mp[2 * blk + j]
        m = dict(W)
        m['XPAD'] = np.ascontiguousarray(xp.reshape(40, 2 * 66 * 66))
        m['STATST'] = np.ascontiguousarray(
            stats[s0:s0 + BS].transpose(2, 0, 1).reshape(7, BS * N))
        m['PERIT'] = np.ascontiguousarray(peri[s0:s0 + BS].T)
        in_maps.append(m)
    res = run_bass_kernel_spmd(nc, in_maps, list(range(NC)))
    v = np.concatenate([res.results[c]['OV'] for c in range(NC)], 0).astype(np.float32)
    a = np.concatenate([res.results[c]['OA'] for c in range(NC)], 0).astype(np.float32)
    return v, a
